# revision 9
# baseline (speedup 1.0000x reference)
"""Switch-Transformer top-1 MoE FFN on 8 Trainium2 NeuronCores.

Strategy (expert parallelism):
  - Router (x @ Wg + bg, softmax, argmax) runs on host CPU jax so routing
    decisions bit-match a CPU-jax reference.
  - The E=4 experts are sharded across the 8 cores: core c serves expert
    c // 2 and gets half of that expert's tokens (token dispatch done on
    host while building the per-core input maps).
  - Each core runs a dense FFN over its padded token slab in bf16:
        hT = gelu(W1.T @ xT + b1)     [d_ff,    C]
        yT = W2.T @ hT + b2           [d_model, C]
    Keeping everything transposed (tokens on the free dim) means both
    matmuls consume the previous result directly -- no on-device
    transposes.
  - Host scatters each core's yT back to token positions and applies the
    winning-route gate scale in fp32.
"""

import numpy as np

D_MODEL = 768
D_FF = 3072
N_EXPERTS = 4
N_CORES = 8
P = 128
TC = 512  # token chunk = matmul free dim = one PSUM bank of fp32
KD = D_MODEL // P  # 6  k-tiles over d_model
KF = D_FF // P  # 24 k-tiles over d_ff

_prog_cache: dict[int, object] = {}


def _make_tile_context(nc):
    """TileContext whose kernel-tail drain carries at most one sync wait.

    This container's walrus rejects Drain instructions (TPB CTRL class)
    with more than one sync-wait command, and stock Tile aggregates every
    outstanding semaphore onto a single tail drain. Emit one drain per
    semaphore wait instead -- semantically identical (all execute on SyncE
    in order before the end-of-kernel barrier).
    """
    import concourse.tile as tile
    from concourse.vector_clock import ScopedClock

    class SplitDrainTileContext(tile.TileContext):
        def _drain_and_barrier(self, tick_clock, wait_clock):
            drain_inst = self.nc.sync.drain()
            wait_clock.add_sem_waits(
                drain_inst.ins, ScopedClock({None: tick_clock.global_clock})
            )
            from concourse import mybir as _mybir

            waits = list(drain_inst.ins.sync_info.on_wait)
            if len(waits) > 1:
                si = drain_inst.ins.sync_info
                si.on_wait = waits[:1]
                for w in waits[1:]:
                    d2 = self.nc.sync.drain()
                    d2.ins.sync_info = _mybir.SyncInfo(on_wait=[w], on_update=[])
            self.nc.all_engine_barrier()
            assert self.sems is not None
            popped = self.nc._tile_sem_poison_stack.pop()
            assert popped is self._sem_poison
            self.nc.clear_and_free_semaphores(list(self.sems.allocated().values()))
            self.nc.all_engine_barrier()

    return SplitDrainTileContext(nc)


def _split_excess_waits(bir_bytes: bytes) -> bytes:
    """Rewrite serialized BIR so no instruction carries more than one sync
    wait: this container's walrus codegen rejects multi-wait instructions
    across TPB instruction classes. Excess waits move to freshly inserted
    same-engine Drain instructions immediately before the consumer, which
    is semantically identical (the engine satisfies them in order)."""
    import json

    d = json.loads(bir_bytes)
    n = 0
    for fn in d["functions"]:
        for blk in fn.get("instruction_blocks") or fn.get("blocks") or []:
            out_list = []
            for ins in blk["instructions"]:
                si = ins.get("sync_info") or {}
                ow = si.get("on_wait") or []
                if len(ow) > 1:
                    for w in ow[:-1]:
                        n += 1
                        out_list.append(
                            {
                                "debug": ins.get("debug", 0),
                                "engine": ins["engine"],
                                "ins": [],
                                "name": f"WSPLIT-{n}",
                                "opcode": "Drain",
                                "outs": [],
                                "sync_info": {"on_update": [], "on_wait": [w]},
                            }
                        )
                    si["on_wait"] = [ow[-1]]
                out_list.append(ins)
            blk["instructions"] = out_list
    return json.dumps(d).encode()


def _install_wait_split(nc):
    orig = nc.to_json_bytes

    def patched():
        return _split_excess_waits(orig())

    nc.to_json_bytes = patched
    return nc


def build_ffn_program(C: int, act: str = "Gelu_apprx_tanh"):
    """Per-core dense expert-FFN Bass program for a [C, D_MODEL] token slab."""
    import concourse.bass as bass
    import concourse.tile as tile
    from concourse import mybir

    if (C, act) in _prog_cache:
        return _prog_cache[(C, act)]

    nc = bass.Bass()
    xT = nc.dram_tensor("xT", [D_MODEL, C], mybir.dt.bfloat16, kind="ExternalInput")
    w1 = nc.dram_tensor("w1", [D_MODEL, D_FF], mybir.dt.bfloat16, kind="ExternalInput")
    w2 = nc.dram_tensor("w2", [D_FF, D_MODEL], mybir.dt.bfloat16, kind="ExternalInput")
    b1 = nc.dram_tensor("b1", [P, KF], mybir.dt.float32, kind="ExternalInput")
    b2 = nc.dram_tensor("b2", [P, KD], mybir.dt.float32, kind="ExternalInput")
    yT = nc.dram_tensor("yT", [D_MODEL, C], mybir.dt.float32, kind="ExternalOutput")

    gelu = getattr(mybir.ActivationFunctionType, act)

    with _make_tile_context(nc) as tc:
        with (
            tc.tile_pool(name="const", bufs=1) as const,
            tc.tile_pool(name="xt", bufs=3) as xpool,
            tc.tile_pool(name="ht", bufs=2) as hpool,
            tc.tile_pool(name="yt", bufs=4) as ypool,
            tc.tile_pool(name="ps1", bufs=4, space="PSUM") as ps1,
            tc.tile_pool(name="ps2", bufs=2, space="PSUM") as ps2,
        ):
            w1t = const.tile([P, KD, D_FF], mybir.dt.bfloat16)
            w2t = const.tile([P, KF, D_MODEL], mybir.dt.bfloat16)
            b1t = const.tile([P, KF], mybir.dt.float32)
            b2t = const.tile([P, KD], mybir.dt.float32)
            for k in range(KD):
                nc.sync.dma_start(out=w1t[:, k, :], in_=w1[k * P : (k + 1) * P, :])
            for k in range(KF):
                nc.sync.dma_start(out=w2t[:, k, :], in_=w2[k * P : (k + 1) * P, :])
            nc.sync.dma_start(out=b1t[:], in_=b1[:])
            nc.sync.dma_start(out=b2t[:], in_=b2[:])

            for ch in range(C // TC):
                cs = slice(ch * TC, (ch + 1) * TC)
                xt = xpool.tile([P, KD, TC], mybir.dt.bfloat16, tag="xt")
                for k in range(KD):
                    nc.sync.dma_start(out=xt[:, k, :], in_=xT[k * P : (k + 1) * P, cs])
                ht = hpool.tile([P, KF, TC], mybir.dt.bfloat16, tag="ht")
                for mf in range(KF):
                    p1 = ps1.tile([P, TC], mybir.dt.float32, tag="p1")
                    for k in range(KD):
                        nc.tensor.matmul(
                            p1[:],
                            w1t[:, k, mf * P : (mf + 1) * P],
                            xt[:, k, :],
                            start=(k == 0),
                            stop=(k == KD - 1),
                        )
                    nc.scalar.activation(
                        ht[:, mf, :], p1[:], gelu, bias=b1t[:, mf : mf + 1]
                    )
                for mo in range(KD):
                    p2 = ps2.tile([P, TC], mybir.dt.float32, tag="p2")
                    for k in range(KF):
                        nc.tensor.matmul(
                            p2[:],
                            w2t[:, k, mo * P : (mo + 1) * P],
                            ht[:, k, :],
                            start=(k == 0),
                            stop=(k == KF - 1),
                        )
                    yt = ypool.tile([P, TC], mybir.dt.float32, tag="yt")
                    nc.vector.tensor_scalar_add(yt[:], p2[:], b2t[:, mo : mo + 1])
                    nc.sync.dma_start(out=yT[mo * P : (mo + 1) * P, cs], in_=yt[:])

    _install_wait_split(nc)
    _prog_cache[(C, act)] = nc
    return nc


def route_tokens(x_flat, Wg, bg):
    """Router on host CPU jax (matches a CPU-jax reference bit-for-bit)."""
    import jax
    import jax.numpy as jnp

    cpu = jax.devices("cpu")[0]
    with jax.default_device(cpu):
        logits = (
            jnp.asarray(x_flat, jnp.float32) @ jnp.asarray(Wg, jnp.float32)
        ) + jnp.asarray(bg, jnp.float32)
        probs = jax.nn.softmax(logits, axis=-1)
        gate = np.asarray(jnp.max(probs, axis=-1))
        route = np.asarray(jnp.argmax(probs, axis=-1))
    return gate, route


def shard_tokens(route):
    """Core c serves expert c // 2; the expert's tokens are split in half."""
    core_idx = []
    for e in range(N_EXPERTS):
        ie = np.nonzero(route == e)[0]
        h = (len(ie) + 1) // 2
        core_idx.append(ie[:h])
        core_idx.append(ie[h:])
    return core_idx


def kernel(hidden_states, Wg, bg, W1, b1, W2, b2):
    import ml_dtypes
    from concourse.bass_utils import run_bass_kernel_spmd

    bf16 = ml_dtypes.bfloat16

    x = np.asarray(hidden_states, np.float32)
    B, S, D = x.shape
    x_flat = x.reshape(-1, D)
    Wg = np.asarray(Wg, np.float32)
    bg = np.asarray(bg, np.float32)
    W1 = np.asarray(W1, np.float32)
    b1 = np.asarray(b1, np.float32)
    W2 = np.asarray(W2, np.float32)
    b2 = np.asarray(b2, np.float32)

    gate, route = route_tokens(x_flat, Wg, bg)
    core_idx = shard_tokens(route)
    cnt = [len(ci) for ci in core_idx]
    C = max(TC, -(-max(cnt) // TC) * TC)

    nc = build_ffn_program(C)

    in_maps = []
    for c in range(N_CORES):
        e = c // 2
        xs = np.zeros((C, D), np.float32)
        xs[: cnt[c]] = x_flat[core_idx[c]]
        in_maps.append(
            {
                "xT": np.ascontiguousarray(xs.T.astype(bf16)),
                "w1": np.ascontiguousarray(W1[e].astype(bf16)),
                "w2": np.ascontiguousarray(W2[e].astype(bf16)),
                "b1": np.ascontiguousarray(b1[e].reshape(KF, P).T),
                "b2": np.ascontiguousarray(b2[e].reshape(KD, P).T),
            }
        )

    results = run_bass_kernel_spmd(nc, in_maps, list(range(N_CORES))).results

    out = np.zeros_like(x_flat)
    for c in range(N_CORES):
        yTc = np.asarray(results[c]["yT"], np.float32)  # [D_MODEL, C]
        idx = core_idx[c]
        out[idx] = yTc.T[: cnt[c]] * gate[idx][:, None]
    return out.reshape(B, S, D)


# revision 23
# speedup vs baseline: 382.6734x; 382.6734x over previous
"""Switch-Transformer top-1 MoE FFN on 8 Trainium2 NeuronCores.

Strategy (expert parallelism):
  - Router (x @ Wg + bg, softmax, argmax) runs on host CPU jax so routing
    decisions bit-match a CPU-jax reference.
  - The E=4 experts are sharded across the 8 cores: core c serves expert
    c // 2 and gets half of that expert's tokens (token dispatch done on
    host while building the per-core input maps).
  - Each core runs a dense FFN over its padded token slab in bf16:
        hT = gelu(W1.T @ xT + b1)     [d_ff,    C]
        yT = W2.T @ hT + b2           [d_model, C]
    Keeping everything transposed (tokens on the free dim) means both
    matmuls consume the previous result directly -- no on-device
    transposes.
  - Host scatters each core's yT back to token positions and applies the
    winning-route gate scale in fp32.
"""

import numpy as np

D_MODEL = 768
D_FF = 3072
N_EXPERTS = 4
N_CORES = 8
P = 128
TC = 512  # token chunk = matmul free dim = one PSUM bank of fp32
KD = D_MODEL // P  # 6  k-tiles over d_model
KF = D_FF // P  # 24 k-tiles over d_ff

_prog_cache: dict[int, object] = {}


def _make_tile_context(nc):
    """TileContext whose kernel-tail drain carries at most one sync wait.

    This container's walrus rejects Drain instructions (TPB CTRL class)
    with more than one sync-wait command, and stock Tile aggregates every
    outstanding semaphore onto a single tail drain. Emit one drain per
    semaphore wait instead -- semantically identical (all execute on SyncE
    in order before the end-of-kernel barrier).
    """
    import concourse.tile as tile
    from concourse.vector_clock import ScopedClock

    class SplitDrainTileContext(tile.TileContext):
        def _drain_and_barrier(self, tick_clock, wait_clock):
            drain_inst = self.nc.sync.drain()
            wait_clock.add_sem_waits(
                drain_inst.ins, ScopedClock({None: tick_clock.global_clock})
            )
            from concourse import mybir as _mybir

            waits = list(drain_inst.ins.sync_info.on_wait)
            if len(waits) > 1:
                si = drain_inst.ins.sync_info
                si.on_wait = waits[:1]
                for w in waits[1:]:
                    d2 = self.nc.sync.drain()
                    d2.ins.sync_info = _mybir.SyncInfo(on_wait=[w], on_update=[])
            self.nc.all_engine_barrier()
            assert self.sems is not None
            popped = self.nc._tile_sem_poison_stack.pop()
            assert popped is self._sem_poison
            self.nc.clear_and_free_semaphores(list(self.sems.allocated().values()))
            self.nc.all_engine_barrier()

    return SplitDrainTileContext(nc)


def _split_excess_waits(bir_bytes: bytes) -> bytes:
    """Rewrite serialized BIR so no instruction carries more than one sync
    wait: this container's walrus codegen rejects multi-wait instructions
    across TPB instruction classes. Excess waits move to freshly inserted
    same-engine Drain instructions immediately before the consumer, which
    is semantically identical (the engine satisfies them in order)."""
    import json

    d = json.loads(bir_bytes)
    n = 0
    for fn in d["functions"]:
        for blk in fn.get("instruction_blocks") or fn.get("blocks") or []:
            out_list = []
            for ins in blk["instructions"]:
                si = ins.get("sync_info") or {}
                ow = si.get("on_wait") or []
                if len(ow) > 1:
                    for w in ow[:-1]:
                        n += 1
                        out_list.append(
                            {
                                "debug": ins.get("debug", 0),
                                "engine": ins["engine"],
                                "ins": [],
                                "name": f"WSPLIT-{n}",
                                "opcode": "Drain",
                                "outs": [],
                                "sync_info": {"on_update": [], "on_wait": [w]},
                            }
                        )
                    si["on_wait"] = [ow[-1]]
                out_list.append(ins)
            blk["instructions"] = out_list
    return json.dumps(d).encode()


def _install_wait_split(nc):
    orig = nc.to_json_bytes

    def patched():
        return _split_excess_waits(orig())

    nc.to_json_bytes = patched
    return nc


def build_ffn_program(C: int, act: str = "Gelu_apprx_tanh", reps: int = 1, pipe: bool = False):
    """Per-core dense expert-FFN Bass program for a [C, D_MODEL] token slab.

    reps > 1 repeats the whole compute body (same data, same output) inside
    one NEFF -- used only for device-time measurement by differencing.
    pipe: issue chunk ch's second matmul after chunk ch+1's first matmul so
    the PE never waits on the gelu activations at a chunk boundary.
    """
    import concourse.bass as bass
    import concourse.tile as tile
    from concourse import mybir

    if (C, act, reps, pipe) in _prog_cache:
        return _prog_cache[(C, act, reps, pipe)]

    nc = bass.Bass()
    xT = nc.dram_tensor("xT", [D_MODEL, C], mybir.dt.bfloat16, kind="ExternalInput")
    w1 = nc.dram_tensor("w1", [D_MODEL, D_FF], mybir.dt.bfloat16, kind="ExternalInput")
    w2 = nc.dram_tensor("w2", [D_FF, D_MODEL], mybir.dt.bfloat16, kind="ExternalInput")
    b1 = nc.dram_tensor("b1", [P, KF], mybir.dt.float32, kind="ExternalInput")
    b2 = nc.dram_tensor("b2", [P, KD], mybir.dt.float32, kind="ExternalInput")
    yT = nc.dram_tensor("yT", [D_MODEL, C], mybir.dt.float32, kind="ExternalOutput")

    gelu = getattr(mybir.ActivationFunctionType, act)

    with _make_tile_context(nc) as tc:
        with (
            tc.tile_pool(name="const", bufs=1) as const,
            tc.tile_pool(name="xt", bufs=3) as xpool,
            tc.tile_pool(name="ht", bufs=3) as hpool,
            tc.tile_pool(name="yt", bufs=4) as ypool,
            tc.tile_pool(name="ps1", bufs=4, space="PSUM") as ps1,
            tc.tile_pool(name="ps2", bufs=2, space="PSUM") as ps2,
        ):
            w1t = const.tile([P, KD, D_FF], mybir.dt.bfloat16)
            w2t = const.tile([P, KF, D_MODEL], mybir.dt.bfloat16)
            b1t = const.tile([P, KF], mybir.dt.float32)
            b2t = const.tile([P, KD], mybir.dt.float32)
            for k in range(KD):
                nc.sync.dma_start(out=w1t[:, k, :], in_=w1[k * P : (k + 1) * P, :])
            nc.sync.dma_start(out=b1t[:], in_=b1[:])
            nc.sync.dma_start(out=b2t[:], in_=b2[:])

            def load_w2():
                # traced after chunk 0's first-matmul phase so the w2 bulk
                # load queues behind w1 + x(0) and overlaps chunk 0 compute
                for k in range(KF):
                    nc.sync.dma_start(
                        out=w2t[:, k, :], in_=w2[k * P : (k + 1) * P, :]
                    )

            # Full TC-wide chunks plus one narrower tail chunk (C need only
            # be a multiple of 128) -- trims token padding at a small PE
            # efficiency cost on the tail only.
            bounds = []
            off = 0
            while off < C:
                w = min(TC, C - off)
                bounds.append((off, w))
                off += w

            def mm1_phase(off, w):
                cs = slice(off, off + w)
                xt = xpool.tile([P, KD, TC], mybir.dt.bfloat16, tag="xt")
                for k in range(KD):
                    nc.sync.dma_start(
                        out=xt[:, k, :w], in_=xT[k * P : (k + 1) * P, cs]
                    )
                ht = hpool.tile([P, KF, TC], mybir.dt.bfloat16, tag="ht")
                for mf in range(KF):
                    p1 = ps1.tile([P, TC], mybir.dt.float32, tag="p1")
                    for k in range(KD):
                        nc.tensor.matmul(
                            p1[:, :w],
                            w1t[:, k, mf * P : (mf + 1) * P],
                            xt[:, k, :w],
                            start=(k == 0),
                            stop=(k == KD - 1),
                        )
                    nc.scalar.activation(
                        ht[:, mf, :w], p1[:, :w], gelu, bias=b1t[:, mf : mf + 1]
                    )
                return ht

            def mm2_phase(ht, off, w):
                cs = slice(off, off + w)
                for mo in range(KD):
                    p2 = ps2.tile([P, TC], mybir.dt.float32, tag="p2")
                    for k in range(KF):
                        nc.tensor.matmul(
                            p2[:, :w],
                            w2t[:, k, mo * P : (mo + 1) * P],
                            ht[:, k, :w],
                            start=(k == 0),
                            stop=(k == KF - 1),
                        )
                    yt = ypool.tile([P, TC], mybir.dt.float32, tag="yt")
                    nc.vector.tensor_scalar_add(yt[:, :w], p2[:, :w], b2t[:, mo : mo + 1])
                    nc.sync.dma_start(out=yT[mo * P : (mo + 1) * P, cs], in_=yt[:, :w])

            sched = list(bounds) * reps
            if pipe:
                pending = None  # (ht, off, w) whose mm2 is deferred one chunk
                for i, (off, w) in enumerate(sched):
                    ht = mm1_phase(off, w)
                    if i == 0:
                        load_w2()
                    if pending is not None:
                        mm2_phase(*pending)
                    pending = (ht, off, w)
                mm2_phase(*pending)
            else:
                for i, (off, w) in enumerate(sched):
                    ht = mm1_phase(off, w)
                    if i == 0:
                        load_w2()
                    mm2_phase(ht, off, w)

    _install_wait_split(nc)
    _prog_cache[(C, act, reps)] = nc
    return nc


def route_tokens(x_flat, Wg, bg):
    """Router on host CPU jax (matches a CPU-jax reference bit-for-bit)."""
    import jax
    import jax.numpy as jnp

    cpu = jax.devices("cpu")[0]
    with jax.default_device(cpu):
        logits = (
            jnp.asarray(x_flat, jnp.float32) @ jnp.asarray(Wg, jnp.float32)
        ) + jnp.asarray(bg, jnp.float32)
        probs = jax.nn.softmax(logits, axis=-1)
        gate = np.asarray(jnp.max(probs, axis=-1))
        route = np.asarray(jnp.argmax(probs, axis=-1))
    return gate, route


def shard_tokens(route):
    """Core c serves expert c // 2; the expert's tokens are split in half."""
    core_idx = []
    for e in range(N_EXPERTS):
        ie = np.nonzero(route == e)[0]
        h = (len(ie) + 1) // 2
        core_idx.append(ie[:h])
        core_idx.append(ie[h:])
    return core_idx


def kernel(hidden_states, Wg, bg, W1, b1, W2, b2):
    import ml_dtypes
    from concourse.bass_utils import run_bass_kernel_spmd

    bf16 = ml_dtypes.bfloat16

    x = np.asarray(hidden_states, np.float32)
    B, S, D = x.shape
    x_flat = x.reshape(-1, D)
    Wg = np.asarray(Wg, np.float32)
    bg = np.asarray(bg, np.float32)
    W1 = np.asarray(W1, np.float32)
    b1 = np.asarray(b1, np.float32)
    W2 = np.asarray(W2, np.float32)
    b2 = np.asarray(b2, np.float32)

    gate, route = route_tokens(x_flat, Wg, bg)
    core_idx = shard_tokens(route)
    cnt = [len(ci) for ci in core_idx]
    C = max(P, -(-max(cnt) // P) * P)

    nc = build_ffn_program(C)

    in_maps = []
    for c in range(N_CORES):
        e = c // 2
        xs = np.zeros((C, D), np.float32)
        xs[: cnt[c]] = x_flat[core_idx[c]]
        in_maps.append(
            {
                "xT": np.ascontiguousarray(xs.T.astype(bf16)),
                "w1": np.ascontiguousarray(W1[e].astype(bf16)),
                "w2": np.ascontiguousarray(W2[e].astype(bf16)),
                "b1": np.ascontiguousarray(b1[e].reshape(KF, P).T),
                "b2": np.ascontiguousarray(b2[e].reshape(KD, P).T),
            }
        )

    results = run_bass_kernel_spmd(nc, in_maps, list(range(N_CORES))).results

    out = np.zeros_like(x_flat)
    for c in range(N_CORES):
        yTc = np.asarray(results[c]["yT"], np.float32)  # [D_MODEL, C]
        idx = core_idx[c]
        out[idx] = yTc.T[: cnt[c]] * gate[idx][:, None]
    return out.reshape(B, S, D)
